# revision 21
# baseline (speedup 1.0000x reference)
"""Bass/Trainium2 kernel for nn_DeConv2d_17136919511113.

Per-(oC,iC)-pair 3-layer MLP (1->16->16->4) applied per pixel, summed over iC,
assembled into a 2x-upsampled image.  Sharding: data-parallel over batch n
(core c handles batch image c).

Pipeline per core (N = 64*64 = 4096 pixels, 8 chunks of 512):
  h1'[o,i,h,p] = max(W1*x, -b1)        one DVE/Pool bf16 4x op (b1 folded into
                                       evac bias: b2' = b2 + W2 @ b1)
  z2 = W2 @ h1'                        PE, 32x32 tile_position matmuls (bf16)
  h2 = relu(z2 + b2')                  ACT/DVE per-bank PSUM->SBUF evac w/ bias
  y  = W3stack @ h2 (K=256 per o)      PE, zero-padded 32x32 tiles -> 4-bank acc
  yo = reduce-sum over 4 banks         DVE tensor_reduce (b3 added on host)
  out: 1 DMA per 512-px chunk to y[128, NPX]; host permutes to upsampled image.
"""
import sys

sys.path.insert(0, "/opt/trn_rl_repo")

import numpy as np
import ml_dtypes

OC, IC, KH, KW, HID = 16, 16, 2, 2, 16
KK = KH * KW
N_CORES = 8
IH = IW = 64
NPX = IH * IW          # per-core pixels (one batch image)
CHUNK = 512
NCH = NPX // CHUNK     # 8 chunks
QCH = 4                # chunks per h1-production group
QN = NCH // QCH        # 2 groups
BF16 = ml_dtypes.bfloat16

# engine cost model (ns) for static load balancing
DVE_EVAC = 760.0
ACT_EVAC = 686.0

_CACHE = {}


def _strip_pairs(H, s):
    """h1 strip (H, s) holds pairs (i0, i0+1) with i0 = 8*H + 2*s."""
    i0 = 8 * H + 2 * s
    return i0, i0 + 1


def _build_bass():
    import concourse.bass as bass
    import concourse.mybir as mybir
    from concourse import bacc
    from concourse.tile import TileContext

    dt = mybir.dt
    Alu = mybir.AluOpType
    Act = mybir.ActivationFunctionType

    nc = bacc.Bacc(None, target_bir_lowering=False, debug=False)

    xai = nc.declare_dram_parameter("xai", [128, NPX], dt.bfloat16, isOutput=False)
    xbi = nc.declare_dram_parameter("xbi", [128, NPX], dt.bfloat16, isOutput=False)
    w1i = nc.declare_dram_parameter("w1i", [128, 32], dt.float32, isOutput=False)
    nb1i = nc.declare_dram_parameter("nb1i", [128, 32], dt.float32, isOutput=False)
    b2i = nc.declare_dram_parameter("b2i", [128, 32], dt.float32, isOutput=False)
    w2i = nc.declare_dram_parameter("w2i", [128, 1024], dt.bfloat16, isOutput=False)
    w3i = nc.declare_dram_parameter("w3i", [128, 1024], dt.bfloat16, isOutput=False)
    # device output layout [p = 32*c3 + 4*g + k, pix]; host permutes + adds b3
    yex = nc.declare_dram_parameter("y", [128, NPX], dt.float32, isOutput=True)

    # engine-balance accounting (ns)
    bal = {"act": 0.0, "dve": 0.0}

    with TileContext(nc) as tc:
        with (
            tc.tile_pool(name="singles", bufs=1) as singles,
            tc.tile_pool(name="h1p", bufs=1) as h1p,
            tc.tile_pool(name="h2p", bufs=8) as h2p,
            tc.tile_pool(name="yp", bufs=2) as yp,
            tc.tile_pool(name="pA", bufs=2, space="PSUM") as pA,
            tc.tile_pool(name="pL3", bufs=1, space="PSUM") as pL3,
        ):
            w1s = singles.tile([128, 32], dt.float32)
            nb1s = singles.tile([128, 32], dt.float32)
            b2s = singles.tile([128, 32], dt.float32)
            w2s = singles.tile([128, 1024], dt.bfloat16)
            w3s = singles.tile([128, 1024], dt.bfloat16)
            x16a = singles.tile([128, NPX], dt.bfloat16)
            x16b = singles.tile([128, NPX], dt.bfloat16)

            nc.gpsimd.dma_start(out=w1s, in_=w1i[:, :])
            nc.gpsimd.dma_start(out=nb1s, in_=nb1i[:, :])
            # x16a rows 16*il + h <- x[il] (host pre-replicated); halves on
            # separate DMA queues so group-0 h1 starts early
            half = QCH * CHUNK
            for q in range(QN):
                qlo = q * half
                for x16, xi in ((x16a, xai), (x16b, xbi)):
                    for piece, eng in ((0, nc.sync), (1, nc.scalar)):
                        plo = qlo + piece * (half // 2)
                        eng.dma_start(out=x16[:, plo : plo + half // 2], in_=xi[:, plo : plo + half // 2])
                if q == 0:
                    nc.gpsimd.dma_start(out=w2s, in_=w2i[:, :])
                else:
                    nc.gpsimd.dma_start(out=b2s, in_=b2i[:, :])
                    nc.gpsimd.dma_start(out=w3s, in_=w3i[:, :])

            h1T = {}
            for o in range(OC):
                for H in (0, 1):
                    h1T[(o, H)] = h1p.tile(
                        [128, QCH * CHUNK], dt.bfloat16,
                        tag=f"h1_{o}_{H}", name=f"h1_{o}_{H}",
                    )

            h2hist = {}
            l3s = {}

            def emit_l3(chunk, grp, betas=(0, 1)):
                # L3 for o-group grp of `chunk`: 32 MMs into 4-bank l3 tile.
                if chunk not in l3s:
                    l3s[chunk] = pL3.tile(
                        [128, 2048], dt.float32, tag="L3", name=f"l3_{chunk}"
                    )
                l3 = l3s[chunk]
                for beta in betas:
                    for c3 in range(4):
                        oo = grp * 4 + c3
                        for r3 in range(4):
                            nc.tensor.matmul(
                                l3[32 * c3 : 32 * c3 + 32, 512 * r3 : 512 * r3 + 512],
                                w3s[32 * r3 : 32 * r3 + 32, (oo * 2 + beta) * 32 : (oo * 2 + beta) * 32 + 32],
                                h2hist[(chunk, oo)][32 * r3 : 32 * r3 + 32, 512 * beta : 512 * beta + 512],
                                start=(grp == 0 and beta == 0),
                                stop=(grp == 3 and beta == 1),
                                tile_position=(32 * r3, 32 * c3),
                            )

            def emit_merge(chunk):
                # merge 2 banks: yo[p, j] = sum_r l3[p, 512 r + j]; b3 on host
                l3 = l3s.pop(chunk)
                yo = yp.tile([128, 512], dt.float32, tag="yo")
                l3v = l3.rearrange("p (r j) -> p j r", r=4)
                nc.vector.tensor_reduce(yo, l3v, mybir.AxisListType.X, Alu.add)
                bal["dve"] += (120 + 2048) / 0.96 + 85
                nc.sync.dma_start(
                    out=yex[:, chunk * CHUNK : (chunk + 1) * CHUNK], in_=yo
                )

            # group-0 h1: H-major so H=0 ops start as soon as x16a lands
            for H, x16 in ((0, x16a), (1, x16b)):
                for o in range(OC):
                    nc.vector.tensor_scalar(
                        h1T[(o, H)],
                        x16[:, 0 : QCH * CHUNK],
                        w1s[:, 2 * o + H : 2 * o + H + 1],
                        nb1s[:, 2 * o + H : 2 * o + H + 1],
                        Alu.mult,
                        Alu.max,
                    )
                    bal["dve"] += (58 + QCH * CHUNK / 4) / 0.96 + 85

            for chunk in range(NCH):
                q, cl = chunk // QCH, chunk % QCH
                qlo = q * QCH * CHUNK
                for o in range(OC):
                    # L2: 8 tile-matmuls -> two per-bank psum tiles [128, 512]
                    # (independent slots -> 4-deep evac pipeline), bank-major
                    pab = {}
                    for bank in (0, 1):
                        pab[bank] = pA.tile(
                            [128, 512], dt.float32, tag=f"A{bank}", name=f"pa{bank}"
                        )
                        for H in (0, 1):
                            for s in (bank, bank + 2):
                                c = (2 * H + s // 2) ^ (o & 1)
                                nc.tensor.matmul(
                                    pab[bank][32 * c : 32 * c + 32, :],
                                    w2s[32 * s : 32 * s + 32, (o * 2 + H) * 32 : (o * 2 + H) * 32 + 32],
                                    h1T[(o, H)][32 * s : 32 * s + 32, cl * CHUNK : cl * CHUNK + CHUNK],
                                    start=True,
                                    stop=True,
                                    tile_position=(32 * s, 32 * c),
                                )
                    # evac: h2 = relu(z2 + b2'), per bank (bias differs per bank).
                    # Static smooth split: h1-heavy chunks (0 and QCH-1, where
                    # DVE runs 32 h1 ops) -> all ACT; else bank0 ACT / bank1 DVE
                    # with 2 extra o's to ACT to cover the reduce.
                    h2 = h2p.tile([128, 1024], dt.bfloat16, tag="h2")
                    h1_heavy = chunk == 0 or (cl == QCH - 1 and q + 1 < QN)
                    for bank in (0, 1):
                        b2col = b2s[:, 2 * o + bank : 2 * o + bank + 1]
                        dst = h2[:, 512 * bank : 512 * bank + 512]
                        src = pab[bank][:, :]
                        to_act = h1_heavy or bank == 0 or o in (5, 11)
                        if to_act:
                            nc.scalar.activation(dst, src, Act.Relu, bias=b2col, scale=1.0)
                            bal["act"] += ACT_EVAC
                        else:
                            nc.vector.tensor_scalar(
                                dst, src, b2col, 0.0, Alu.add, Alu.max
                            )
                            bal["dve"] += DVE_EVAC
                    h2hist[(chunk, o)] = h2
                    if cl == QCH - 1 and q + 1 < QN:
                        nqlo = (q + 1) * QCH * CHUNK
                        for H, x16 in ((0, x16a), (1, x16b)):
                            nc.vector.tensor_scalar(
                                h1T[(o, H)],
                                x16[:, nqlo : nqlo + QCH * CHUNK],
                                w1s[:, 2 * o + H : 2 * o + H + 1],
                                nb1s[:, 2 * o + H : 2 * o + H + 1],
                                Alu.mult,
                                Alu.max,
                            )
                            bal["dve"] += (58 + QCH * CHUNK / 4) / 0.96 + 85
                    # deferred pipeline stages (keep PE/DVE queues unblocked):
                    #  - merge of chunk-1 early in this chunk
                    #  - L3 group g two o-slots after its last evac; g3 of
                    #    chunk-1 lands after this chunk's first L2
                    if chunk > 0:
                        if o == 0:
                            emit_l3(chunk - 1, 3)
                        elif o == 2:
                            emit_merge(chunk - 1)
                    if o >= 5 and (o - 5) % 4 == 0 and o < OC - 2:
                        emit_l3(chunk, (o - 5) // 4, betas=(0,))
                    if o >= 6 and (o - 6) % 4 == 0 and o < OC - 1:
                        emit_l3(chunk, (o - 6) // 4, betas=(1,))
            emit_l3(NCH - 1, 3)
            emit_merge(NCH - 1)

    nc.compile()
    return nc


def _prep_weights(W1, b1, W2, b2, W3, b3):
    """Host-side packing of weights into SBUF-image layouts (shared by all cores)."""
    w1i = np.zeros((128, 32), np.float32)
    nb1i = np.zeros((128, 32), np.float32)
    b2i = np.zeros((128, 32), np.float32)
    w2i = np.zeros((128, 1024), np.float32)
    w3i = np.zeros((128, 1024), np.float32)
    # b2' = b2 + W2 @ b1 (per (o,i,g)): compensates h1' = relu(W1 x + b1) - b1
    b2p = b2 + np.einsum("oigh,oih->oig", W2, b1)
    for o in range(OC):
        for H in (0, 1):
            # h1 group H rows: 16*il + h  -> i = 8H + il
            w1i[:, 2 * o + H] = W1[o, 8 * H : 8 * H + 8, :].reshape(128)
            nb1i[:, 2 * o + H] = -b1[o, 8 * H : 8 * H + 8, :].reshape(128)
        # L2 lhsT tiles: strip (H, s) at partitions [32s..], col block (o*2+H)
        for H in (0, 1):
            for s in range(4):
                i0, i1 = _strip_pairs(H, s)
                blk = np.zeros((32, 32), np.float32)
                blk[0:16, 0:16] = W2[o, i0].T      # lhsT[h, g] = W2[g, h]
                blk[16:32, 16:32] = W2[o, i1].T
                w2i[32 * s : 32 * s + 32, (o * 2 + H) * 32 : (o * 2 + H) * 32 + 32] = blk
        # b2 evac bias: psum block c holds strip decoded from c ^ (o&1)
        for bank in (0, 1):
            col = np.zeros(128, np.float32)
            for c in range(4):
                cc = c ^ (o & 1)
                H = cc // 2
                s = 2 * (cc % 2) + bank
                i0, i1 = _strip_pairs(H, s)
                col[32 * c : 32 * c + 16] = b2p[o, i0]
                col[32 * c + 16 : 32 * c + 32] = b2p[o, i1]
            b2i[:, 2 * o + bank] = col
        # L3 lhsT tiles: h2 partition block r3 holds strip decoded from
        # r3 ^ (o&1) (odd o uses complemented L2 psum positions)
        grp = o // 4
        for beta in (0, 1):
            for r3 in range(4):
                rr = r3 ^ (o & 1)
                H = rr // 2
                s = 2 * (rr % 2) + beta
                i0, i1 = _strip_pairs(H, s)
                blk = np.zeros((32, 32), np.float32)
                # rows: (i0 g 0..15, i1 g 16..31); cols 4*grp + k = W3[o, i, k, g]
                blk[0:16, 4 * grp : 4 * grp + 4] = W3[o, i0].T   # [g, k]
                blk[16:32, 4 * grp : 4 * grp + 4] = W3[o, i1].T
                w3i[32 * r3 : 32 * r3 + 32, (o * 2 + beta) * 32 : (o * 2 + beta) * 32 + 32] = blk
    b3sum = b3.sum(axis=1)  # [oC, KK]
    b3i = np.zeros((128, 1), np.float32)
    for c3 in range(4):
        for g in range(4):
            for k in range(KK):
                b3i[32 * c3 + 4 * g + k, 0] = b3sum[4 * g + c3, k]
    return {
        "w1i": w1i,
        "nb1i": nb1i,
        "b2i": b2i,
        "w2i": w2i.astype(BF16),
        "w3i": w3i.astype(BF16),
    }, b3i


def kernel(batches, W1, b1, W2, b2, W3, b3):
    from concourse.bass_utils import run_bass_kernel_spmd

    if "nc" not in _CACHE:
        _CACHE["nc"] = _build_bass()
    nc = _CACHE["nc"]

    wmaps, b3i = _prep_weights(
        np.asarray(W1, np.float32), np.asarray(b1, np.float32),
        np.asarray(W2, np.float32), np.asarray(b2, np.float32),
        np.asarray(W3, np.float32), np.asarray(b3, np.float32),
    )
    batches = np.asarray(batches, np.float32)
    n = batches.shape[0]
    assert n == N_CORES
    in_maps = []
    for cidx in range(N_CORES):
        xr = batches[cidx].reshape(IC, 1, NPX).astype(BF16)
        xr = np.broadcast_to(xr, (IC, HID, NPX)).reshape(2, 128, NPX)
        in_maps.append({"xai": np.ascontiguousarray(xr[0]), "xbi": np.ascontiguousarray(xr[1]), **wmaps})
    res = run_bass_kernel_spmd(nc, in_maps, list(range(N_CORES)))
    out = np.empty((N_CORES, OC, KH * IH, KW * IW), np.float32)
    for cidx in range(N_CORES):
        ydev = res.results[cidx]["y"].astype(np.float32) + b3i
        # partition p = 32*c3 + 4*g + k (k = 2*kh + kw, o = 4*g + c3);
        # rows 16..31 of each 32-block are padding
        yd = ydev.reshape(4, 32, IH, IW)[:, :16]
        yd = yd.reshape(4, 4, KH, KW, IH, IW)          # [c3, g, kh, kw, ih, iw]
        out[cidx] = yd.transpose(1, 0, 4, 2, 5, 3).reshape(OC, KH * IH, KW * IW)
    return out
